# revision 10
# baseline (speedup 1.0000x reference)
"""Trainium2 Bass kernel for nn_AugementationAttention.

Reference computation (per batch b of 16, N=512, D=4096, NH=16, DK=256):
    q = x @ Wq.T, k = x @ Wk.T, v = x @ Wv.T          (per-head dk=256)
    ep = aug @ Wp.T + bp                               (per-head n=512 == 2*dk)
    dist = softmax(q k^T / sqrt(dk) + ep, axis=-1)
    out  = dist @ v                                    -> (b, n, d)

Sharding: data-parallel over batch, 2 batch elements per core on 8 cores.

Per-core kernel structure (single fused pass over (batch, head)):
  - x^T and aug^T for the core's 2 batches stay resident in SBUF (16 MB).
  - Per head: stream this head's weight slices from HBM once, compute
    Q^T/K^T (dout-major) and V (n-major) projections as three separate
    32-k-tile accumulation sub-loops (so only 2 PSUM banks are live per
    sub-loop), then accumulate S^T[key, query] = (aug Wp^T)^T + (QK^T)^T
    in 4 PSUM banks, exp on ScalarE with the bp bias folded in.
  - The A@V stage of head h is deferred until after head h+1's projection
    matmuls (software pipelining) so the PE never waits on ScalarE's exp.
  - A ones-column appended to V makes column 256 of the A@V PSUM tile the
    softmax denominator; normalization happens in the PSUM->SBUF copy
    (vector reciprocal + per-partition tensor_scalar multiply).
  - Scores stay in the transposed [key, query] layout throughout, so no
    on-chip transposes are needed anywhere.
  - All matmuls run as float32r (full PE rate for free dims >= 256).
"""

import sys

sys.path.insert(0, "/opt/trn_rl_repo")

import numpy as np

import concourse.bacc as bacc
import concourse.mybir as mybir
import concourse.tile as tile
from concourse.bass_utils import run_bass_kernel_spmd

F32 = mybir.dt.float32
F32R = mybir.dt.float32r

B, N, D, NH, DK = 16, 512, 4096, 16, 256
NCORES = 8
BL = B // NCORES  # batches per core
KT = D // 128  # 32 k-tiles of the contraction dim
G = 4  # k-tiles per weight DMA chunk
NG = KT // G  # chunks per head per projection
SCALE = 1.0 / np.sqrt(DK)

def _build_program():
    nc = bacc.Bacc(
        "TRN2",
        target_bir_lowering=False,
        debug=False,
        enable_asserts=False,
        num_devices=NCORES,
    )

    xt = nc.dram_tensor("xt", [BL, 128, KT, N], F32R, kind="ExternalInput")
    at = nc.dram_tensor("at", [BL, 128, KT, N], F32R, kind="ExternalInput")
    # w*[h, g, p, G*dout]: per chunk g, k-tile j in chunk, dout cols of head h
    wq = nc.dram_tensor("wq", [NH, NG, 128, G * 256], F32R, kind="ExternalInput")
    wk = nc.dram_tensor("wk", [NH, NG, 128, G * 256], F32R, kind="ExternalInput")
    wv = nc.dram_tensor("wv", [NH, NG, 128, G * 256], F32R, kind="ExternalInput")
    wp = nc.dram_tensor("wp", [NH, NG, 128, G * 512], F32R, kind="ExternalInput")
    bias = nc.dram_tensor("bias", [128, 64], F32, kind="ExternalInput")
    out = nc.dram_tensor("out", [BL, N, D], F32, kind="ExternalOutput")

    with tile.TileContext(nc) as tc:
        with (
            tc.tile_pool(name="const", bufs=1) as const_pool,
            tc.tile_pool(name="acts", bufs=1) as act_pool,
            tc.tile_pool(name="wgt", bufs=2) as w_pool,
            tc.tile_pool(name="qk", bufs=1) as qk_pool,
            tc.tile_pool(name="vv", bufs=2) as v_pool,
            tc.tile_pool(name="ee", bufs=2) as e_pool,
            tc.tile_pool(name="oo", bufs=2) as o_pool,
            tc.tile_pool(name="rr", bufs=8) as r_pool,
            tc.tile_pool(name="psp", bufs=4, space="PSUM") as ps_proj,
            tc.tile_pool(name="pss", bufs=4, space="PSUM") as ps_s,
        ):
            bias_sb = const_pool.tile([128, 64], F32)
            nc.sync.dma_start(out=bias_sb[:], in_=bias[:])

            def proj_qk(w_dram, h, xt_sb, name):
                """Q^T/K^T projection: psum[dt] [128 dout, N] over 32 k-tiles."""
                ps = [
                    ps_proj.tile([128, N], F32, tag="pp", name=f"ps{name}{i}")
                    for i in range(2)
                ]
                for g in range(NG):
                    wt = w_pool.tile([128, G, 256], F32R, tag="wqk", name=f"w{name}")
                    eng = nc.sync if g % 2 == 0 else nc.scalar
                    eng.dma_start(out=wt[:], in_=w_dram[h, g])
                    for j in range(G):
                        kt = g * G + j
                        st, sp = kt == 0, kt == KT - 1
                        xk = (xt_sb[:, kt, :])
                        for dt in range(2):
                            nc.tensor.matmul(
                                ps[dt][:],
                                (wt[:, j, dt * 128 : (dt + 1) * 128]),
                                xk,
                                start=st,
                                stop=sp,
                            )
                sb = qk_pool.tile([128, 2, N], F32R, tag=f"{name}t", name=f"{name}t_sb")
                for dt in range(2):
                    nc.vector.tensor_copy(sb[:, dt, :], ps[dt][:])
                return sb

            def proj_v(h, xt_sb):
                """V projection (n-major): psum tiles pack 2 n-tiles per bank."""
                ps = [
                    ps_proj.tile([128, N], F32, tag="pp", name=f"psv{i}")
                    for i in range(2)
                ]
                for g in range(NG):
                    wt = w_pool.tile([128, G, 256], F32R, tag="wqk", name="wv")
                    eng = nc.sync if g % 2 == 0 else nc.scalar
                    eng.dma_start(out=wt[:], in_=wv[h, g])
                    for j in range(G):
                        kt = g * G + j
                        wvj = (wt[:, j, :])
                        for nt in range(4):
                            # one accumulation group per PSUM bank: start only
                            # zeroes the whole 2KB zero region once, stop on the
                            # bank's last matmul
                            nc.tensor.matmul(
                                ps[nt // 2][:, (nt % 2) * 256 : (nt % 2 + 1) * 256],
                                (xt_sb[:, kt, nt * 128 : (nt + 1) * 128]),
                                wvj,
                                start=(kt == 0 and nt % 2 == 0),
                                stop=(kt == KT - 1 and nt % 2 == 1),
                            )
                v_sb = v_pool.tile([128, 4, 264], F32R, tag="v", name="v_sb")
                for nt in range(4):
                    nc.vector.tensor_copy(
                        v_sb[:, nt, 0:256],
                        ps[nt // 2][:, (nt % 2) * 256 : (nt % 2 + 1) * 256],
                    )
                nc.vector.memset(v_sb[:, :, 256:257].bitcast(F32), 1.0)
                nc.vector.memset(v_sb[:, :, 257:258].bitcast(F32), 0.0)
                return v_sb

            def attn_out(state):
                """Deferred A@V + normalize + store for a previous head."""
                if state is None:
                    return
                b, h, e_sb, v_sb = state
                ot_sb = o_pool.tile([128, 4, 256], F32, tag="ot", name="ot_sb")
                for qt in range(4):
                    pso = ps_s.tile([128, 258], F32, tag="ss", name="pso")
                    for jt in range(4):
                        nc.tensor.matmul(
                            pso[:],
                            (e_sb[:, jt, qt * 128 : (qt + 1) * 128]),
                            (v_sb[:, jt, 0:258]),
                            start=(jt == 0),
                            stop=(jt == 3),
                        )
                    r = r_pool.tile([128, 1], F32, tag="r", name="r")
                    nc.vector.reciprocal(r[:], pso[:, 256:257])
                    nc.vector.tensor_scalar_mul(ot_sb[:, qt, :], pso[:, 0:256], r[:])
                nc.gpsimd.dma_start(
                    out=out[b].rearrange("(qt p) d -> p qt d", p=128)[
                        :, :, h * 256 : (h + 1) * 256
                    ],
                    in_=ot_sb[:],
                )

            pending = None
            for b in range(BL):
                xt_sb = act_pool.tile([128, KT, N], F32R, tag="xt", name="xt_sb")
                nc.sync.dma_start(out=xt_sb[:], in_=xt[b])
                at_sb = act_pool.tile([128, KT, N], F32R, tag="at", name="at_sb")
                nc.scalar.dma_start(out=at_sb[:], in_=at[b])

                for h in range(NH):
                    qt_sb = proj_qk(wq, h, xt_sb, "q")
                    kt_sb = proj_qk(wk, h, xt_sb, "k")
                    v_sb = proj_v(h, xt_sb)

                    # previous head's A@V runs here: by now ScalarE has had a
                    # whole projection phase to finish the previous exp.
                    attn_out(pending)

                    # S^T[j, q] accumulation: Wp-projection + QK^T
                    pss = [
                        ps_s.tile([128, N], F32, tag="ss", name=f"pss{i}")
                        for i in range(4)
                    ]
                    for g in range(NG):
                        wt = w_pool.tile([128, G, 512], F32R, tag="wp", name="wpt")
                        eng = nc.sync if g % 2 == 0 else nc.scalar
                        eng.dma_start(out=wt[:], in_=wp[h, g])
                        for j in range(G):
                            kt = g * G + j
                            ak = (at_sb[:, kt, :])
                            for jt in range(4):
                                nc.tensor.matmul(
                                    pss[jt][:],
                                    (wt[:, j, jt * 128 : (jt + 1) * 128]),
                                    ak,
                                    start=(kt == 0),
                                    stop=False,
                                )
                    for jt in range(4):
                        for dt in range(2):
                            nc.tensor.matmul(
                                pss[jt][:],
                                (kt_sb[:, dt, jt * 128 : (jt + 1) * 128]),
                                (qt_sb[:, dt, :]),
                                start=False,
                                stop=(dt == 1),
                            )

                    e_sb = e_pool.tile([128, 4, N], F32R, tag="e", name="e_sb")
                    for jt in range(4):
                        nc.scalar.activation(
                            e_sb[:, jt, :],
                            pss[jt][:],
                            mybir.ActivationFunctionType.Exp,
                            bias=bias_sb[:, h * 4 + jt : h * 4 + jt + 1],
                        )
                    pending = (b, h, e_sb, v_sb)

            attn_out(pending)

    nc.compile()
    return nc


_NC_CACHE = None


def _get_program():
    global _NC_CACHE
    if _NC_CACHE is None:
        _NC_CACHE = _build_program()
    return _NC_CACHE


def _pack_inputs(x, Augementation_embedding, Wq, Wk, Wv, Wp, bp):
    """Host-side relayout: transposes and per-head tiling, all in numpy."""
    f = np.float32
    x = np.asarray(x, f)
    aug = np.asarray(Augementation_embedding, f)

    # [B, N, D] -> [B, 128, KT, N] : k-tiled transpose
    def act_pack(a):
        t = a.transpose(0, 2, 1).reshape(B, KT, 128, N).transpose(0, 2, 1, 3)
        return np.ascontiguousarray(t)

    xt = act_pack(x)
    at = act_pack(aug)

    # W.T [D, dout_total] -> [NH, NG, 128, G*dout_per_head]
    def w_pack(w_t, dout_per_head):
        nh = w_t.shape[1] // dout_per_head
        t = w_t.reshape(KT, 128, nh, dout_per_head).transpose(2, 0, 1, 3)
        t = (
            t.reshape(nh, NG, G, 128, dout_per_head)
            .transpose(0, 1, 3, 2, 4)
            .reshape(nh, NG, 128, G * dout_per_head)
        )
        return np.ascontiguousarray(t)

    wq_pk = w_pack(np.asarray(Wq, f).T * np.float32(SCALE), 256)
    wk_pk = w_pack(np.asarray(Wk, f).T, 256)
    wv_pk = w_pack(np.asarray(Wv, f).T, 256)
    wp_pk = w_pack(np.asarray(Wp, f).T, 512)

    bias = np.ascontiguousarray(np.asarray(bp, f).reshape(64, 128).T)  # [128, 64]

    return xt, at, wq_pk, wk_pk, wv_pk, wp_pk, bias


def kernel(x, Augementation_embedding, Wq, Wk, Wv, Wp, bp):
    nc = _get_program()
    xt, at, wq_pk, wk_pk, wv_pk, wp_pk, bias = _pack_inputs(
        x, Augementation_embedding, Wq, Wk, Wv, Wp, bp
    )

    in_maps = []
    for c in range(NCORES):
        in_maps.append(
            {
                "xt": xt[c * BL : (c + 1) * BL],
                "at": at[c * BL : (c + 1) * BL],
                "wq": wq_pk,
                "wk": wk_pk,
                "wv": wv_pk,
                "wp": wp_pk,
                "bias": bias,
            }
        )

    res = run_bass_kernel_spmd(nc, in_maps, core_ids=list(range(NCORES)))
    outs = [res.results[c]["out"] for c in range(NCORES)]
    return np.concatenate(outs, axis=0).astype(np.float32)


if __name__ == "__main__":
    rng = np.random.default_rng(0)
    ins = {
        "x": rng.standard_normal((B, N, D), dtype=np.float32),
        "Augementation_embedding": rng.standard_normal((B, N, D), dtype=np.float32),
        "Wq": rng.standard_normal((D, D), dtype=np.float32) / np.sqrt(D),
        "Wk": rng.standard_normal((D, D), dtype=np.float32) / np.sqrt(D),
        "Wv": rng.standard_normal((D, D), dtype=np.float32) / np.sqrt(D),
        "Wp": rng.standard_normal((2 * D, D), dtype=np.float32) / np.sqrt(D),
        "bp": (rng.standard_normal(2 * D, dtype=np.float32) * 0.01),
    }
    o = kernel(**ins)
    print("out", o.shape, o.dtype, float(np.abs(o).max()))


# revision 11
# speedup vs baseline: 1.3771x; 1.3771x over previous
"""Trainium2 Bass kernel for nn_AugementationAttention.

Reference computation (per batch b of 16, N=512, D=4096, NH=16, DK=256):
    q = x @ Wq.T, k = x @ Wk.T, v = x @ Wv.T          (per-head dk=256)
    ep = aug @ Wp.T + bp                               (per-head n=512 == 2*dk)
    dist = softmax(q k^T / sqrt(dk) + ep, axis=-1)
    out  = dist @ v                                    -> (b, n, d)

Sharding: data-parallel over batch, 2 batch elements per core on 8 cores.

Per-core kernel structure (single fused pass over heads, batches paired):
  - x^T and aug^T for BOTH of the core's batch elements stay resident in
    SBUF as bf16 ([128, 32, 1024], batch elements side by side on the
    free dim), so every weight byte is read from HBM exactly once.
  - Projection matmuls (Q/K/V and the Wp augmentation projection) run in
    bf16 (weights streamed from HBM in bf16 — this halves DMA traffic,
    which is the measured bottleneck at ~220 GB/s/core); everything
    computed on-chip downstream (QK^T, exp, A@V) runs in float32r at
    full PE rate, accumulating in fp32 PSUM.
  - Scores are kept in the transposed [key, query] layout so no on-chip
    transposes are needed; softmax normalization over the partition (key)
    axis uses a ones-column appended to V: column 256 of the A@V PSUM
    tile is the softmax denominator, applied during the PSUM->SBUF copy
    (vector reciprocal + per-partition tensor_scalar multiply). The bp
    bias row folds into ScalarE's exp (bias operand).
  - The A@V stage of head h is deferred until after head h+1's Q
    projection, so the PE never waits on ScalarE's exp.
  - S^T accumulates in PSUM in two jt-halves (4 banks each), alternating
    between the two PSUM tag pools so heads flow without bank stalls.
"""

import sys

sys.path.insert(0, "/opt/trn_rl_repo")

import numpy as np
import ml_dtypes

import concourse.bacc as bacc
import concourse.mybir as mybir
import concourse.tile as tile
from concourse.bass_utils import run_bass_kernel_spmd

F32 = mybir.dt.float32
F32R = mybir.dt.float32r
BF16 = mybir.dt.bfloat16

B, N, D, NH, DK = 16, 512, 4096, 16, 256
NCORES = 8
BL = B // NCORES  # batch elements per core
NB = BL * N  # 1024: both batch elements side by side on the free dim
KT = D // 128  # 32 k-tiles of the contraction dim
G = 4  # k-tiles per weight DMA chunk
NG = KT // G  # chunks per head per projection
SCALE = 1.0 / np.sqrt(DK)


def _build_program():
    nc = bacc.Bacc(
        "TRN2",
        target_bir_lowering=False,
        debug=False,
        enable_asserts=False,
        num_devices=NCORES,
    )

    xt = nc.dram_tensor("xt", [128, KT, NB], BF16, kind="ExternalInput")
    at = nc.dram_tensor("at", [128, KT, NB], BF16, kind="ExternalInput")
    # w*[h, g, p, G*256]: per chunk g, k-tile j in chunk, 256 dout cols
    wq = nc.dram_tensor("wq", [NH, NG, 128, G * 256], BF16, kind="ExternalInput")
    wk = nc.dram_tensor("wk", [NH, NG, 128, G * 256], BF16, kind="ExternalInput")
    wv = nc.dram_tensor("wv", [NH, NG, 128, G * 256], BF16, kind="ExternalInput")
    # wp packed as 32 pseudo-heads of 256 jrow cols: index h*2 + jt_half
    wp = nc.dram_tensor("wp", [2 * NH, NG, 128, G * 256], BF16, kind="ExternalInput")
    bias = nc.dram_tensor("bias", [128, 64], F32, kind="ExternalInput")
    out = nc.dram_tensor("out", [BL, N, D], F32, kind="ExternalOutput")

    with tile.TileContext(nc) as tc:
        with (
            tc.tile_pool(name="const", bufs=1) as const_pool,
            tc.tile_pool(name="acts", bufs=1) as act_pool,
            tc.tile_pool(name="wgt", bufs=3) as w_pool,
            tc.tile_pool(name="qk", bufs=2) as qk_pool,
            tc.tile_pool(name="vv", bufs=2) as v_pool,
            tc.tile_pool(name="ee", bufs=2) as e_pool,
            tc.tile_pool(name="oo", bufs=3) as o_pool,
            tc.tile_pool(name="rr", bufs=8) as r_pool,
            tc.tile_pool(name="psp", bufs=4, space="PSUM") as ps_a,
            tc.tile_pool(name="pss", bufs=4, space="PSUM") as ps_b,
        ):
            bias_sb = const_pool.tile([128, 64], F32)
            nc.sync.dma_start(out=bias_sb[:], in_=bias[:])

            # resident activations, loaded in quarters so compute can start
            # as soon as the first k-tiles land
            xt_sb = act_pool.tile([128, KT, NB], BF16, tag="xt", name="xt_sb")
            at_sb = act_pool.tile([128, KT, NB], BF16, tag="at", name="at_sb")
            for q4 in range(4):
                sl = slice(q4 * (KT // 4), (q4 + 1) * (KT // 4))
                nc.sync.dma_start(out=xt_sb[:, sl, :], in_=xt[:, sl, :])
                nc.scalar.dma_start(out=at_sb[:, sl, :], in_=at[:, sl, :])

            def proj_qk(w_dram, h, name):
                """Q^T/K^T projection for both batches: psum[b][dt] [128, N]."""
                ps = [
                    [
                        ps_a.tile([128, N], F32, tag="pp", name=f"ps{name}{b}{i}")
                        for i in range(2)
                    ]
                    for b in range(BL)
                ]
                for g in range(NG):
                    wt = w_pool.tile([128, G, 256], BF16, tag="wqk", name=f"w{name}")
                    eng = nc.sync if g % 2 == 0 else nc.scalar
                    eng.dma_start(out=wt[:], in_=w_dram[h, g])
                    for j in range(G):
                        kt = g * G + j
                        st, sp = kt == 0, kt == KT - 1
                        for b in range(BL):
                            xk = xt_sb[:, kt, b * N : (b + 1) * N]
                            for dt in range(2):
                                nc.tensor.matmul(
                                    ps[b][dt][:],
                                    wt[:, j, dt * 128 : (dt + 1) * 128],
                                    xk,
                                    start=st,
                                    stop=sp,
                                )
                sbs = []
                for b in range(BL):
                    sb = qk_pool.tile(
                        [128, 2, N], F32R, tag=f"{name}t", name=f"{name}t_sb{b}"
                    )
                    for dt in range(2):
                        nc.vector.tensor_copy(sb[:, dt, :], ps[b][dt][:])
                    sbs.append(sb)
                return sbs

            def proj_v(h):
                """V projection (n-major) for both batches, ones col appended."""
                ps = [
                    [
                        ps_a.tile([128, N], F32, tag="pp", name=f"psv{b}{i}")
                        for i in range(2)
                    ]
                    for b in range(BL)
                ]
                for g in range(NG):
                    wt = w_pool.tile([128, G, 256], BF16, tag="wqk", name="wv")
                    eng = nc.sync if g % 2 == 0 else nc.scalar
                    eng.dma_start(out=wt[:], in_=wv[h, g])
                    for j in range(G):
                        kt = g * G + j
                        wvj = wt[:, j, :]
                        for b in range(BL):
                            for nt in range(4):
                                nc.tensor.matmul(
                                    ps[b][nt // 2][
                                        :, (nt % 2) * 256 : (nt % 2 + 1) * 256
                                    ],
                                    xt_sb[
                                        :, kt, b * N + nt * 128 : b * N + (nt + 1) * 128
                                    ],
                                    wvj,
                                    start=(kt == 0 and nt % 2 == 0),
                                    stop=(kt == KT - 1 and nt % 2 == 1),
                                )
                v_sbs = []
                for b in range(BL):
                    v_sb = v_pool.tile([128, 4, 264], F32R, tag="v", name=f"v_sb{b}")
                    for nt in range(4):
                        nc.vector.tensor_copy(
                            v_sb[:, nt, 0:256],
                            ps[b][nt // 2][:, (nt % 2) * 256 : (nt % 2 + 1) * 256],
                        )
                    nc.vector.memset(v_sb[:, :, 256:257].bitcast(F32), 1.0)
                    nc.vector.memset(v_sb[:, :, 257:258].bitcast(F32), 0.0)
                    v_sbs.append(v_sb)
                return v_sbs

            def attn_out(state):
                """Deferred A@V + normalize + store for the previous head."""
                if state is None:
                    return
                h, e_sbs, v_sbs = state
                for b in range(BL):
                    e_sb, v_sb = e_sbs[b], v_sbs[b]
                    ot_sb = o_pool.tile([128, 4, 256], F32, tag="ot", name="ot_sb")
                    for qt in range(4):
                        pso = ps_b.tile([128, 258], F32, tag="ss", name="pso")
                        for jt in range(4):
                            nc.tensor.matmul(
                                pso[:],
                                e_sb[:, jt, qt * 128 : (qt + 1) * 128],
                                v_sb[:, jt, 0:258],
                                start=(jt == 0),
                                stop=(jt == 3),
                            )
                        r = r_pool.tile([128, 1], F32, tag="r", name="r")
                        nc.vector.reciprocal(r[:], pso[:, 256:257])
                        nc.vector.tensor_scalar_mul(
                            ot_sb[:, qt, :], pso[:, 0:256], r[:]
                        )
                    eng = nc.sync if (h + b) % 2 == 0 else nc.scalar
                    eng.dma_start(
                        out=out[b].rearrange("(qt p) d -> p qt d", p=128)[
                            :, :, h * 256 : (h + 1) * 256
                        ],
                        in_=ot_sb[:],
                    )

            pending = None
            for h in range(NH):
                qt_sbs = proj_qk(wq, h, "q")
                # previous head's A@V runs here: ScalarE has long finished the
                # previous exp, and E/V tiles free up before this head's own
                # S phase needs their pool slots.
                attn_out(pending)
                kt_sbs = proj_qk(wk, h, "k")
                v_sbs = proj_v(h)

                # S^T[j, q] in two jt-halves: Wp-projection + QK^T, then exp
                e_sbs = [
                    e_pool.tile([128, 4, N], F32R, tag="e", name=f"e_sb{b}")
                    for b in range(BL)
                ]
                for half in range(2):
                    pool = ps_b if half == 0 else ps_a
                    ptag = "ss" if half == 0 else "pp"
                    pss = [
                        [
                            pool.tile([128, N], F32, tag=ptag, name=f"pss{b}{i}")
                            for i in range(2)
                        ]
                        for b in range(BL)
                    ]
                    for g in range(NG):
                        wt = w_pool.tile([128, G, 256], BF16, tag="wp", name="wpt")
                        eng = nc.sync if g % 2 == 0 else nc.scalar
                        eng.dma_start(out=wt[:], in_=wp[h * 2 + half, g])
                        for j in range(G):
                            kt = g * G + j
                            for b in range(BL):
                                ak = at_sb[:, kt, b * N : (b + 1) * N]
                                for jt2 in range(2):
                                    nc.tensor.matmul(
                                        pss[b][jt2][:],
                                        wt[:, j, jt2 * 128 : (jt2 + 1) * 128],
                                        ak,
                                        start=(kt == 0),
                                        stop=False,
                                    )
                    for b in range(BL):
                        for jt2 in range(2):
                            jt = half * 2 + jt2
                            for dt in range(2):
                                nc.tensor.matmul(
                                    pss[b][jt2][:],
                                    kt_sbs[b][:, dt, jt * 128 : (jt + 1) * 128],
                                    qt_sbs[b][:, dt, :],
                                    start=False,
                                    stop=(dt == 1),
                                )
                            nc.scalar.activation(
                                e_sbs[b][:, jt, :],
                                pss[b][jt2][:],
                                mybir.ActivationFunctionType.Exp,
                                bias=bias_sb[:, h * 4 + jt : h * 4 + jt + 1],
                            )
                pending = (h, e_sbs, v_sbs)

            attn_out(pending)

    nc.compile()
    return nc


_NC_CACHE = None


def _get_program():
    global _NC_CACHE
    if _NC_CACHE is None:
        _NC_CACHE = _build_program()
    return _NC_CACHE


def _pack_inputs(x, Augementation_embedding, Wq, Wk, Wv, Wp, bp):
    """Host-side relayout: transposes, per-head tiling, bf16 casts."""
    f = np.float32
    bf = ml_dtypes.bfloat16
    x = np.asarray(x, f)
    aug = np.asarray(Augementation_embedding, f)

    # [B, N, D] -> per core [128, KT, 2*N] bf16 (batch pair side by side)
    def act_pack(a):
        t = a.transpose(0, 2, 1).reshape(B, KT, 128, N).transpose(0, 2, 1, 3)
        t = t.reshape(NCORES, BL, 128, KT, N).transpose(0, 2, 3, 1, 4)
        return np.ascontiguousarray(t.reshape(NCORES, 128, KT, BL * N).astype(bf))

    xt = act_pack(x)
    at = act_pack(aug)

    # W.T [D, dout_total] -> [nh, NG, 128, G*256] bf16
    def w_pack(w_t):
        nh = w_t.shape[1] // 256
        t = w_t.reshape(KT, 128, nh, 256).transpose(2, 0, 1, 3)
        t = (
            t.reshape(nh, NG, G, 128, 256)
            .transpose(0, 1, 3, 2, 4)
            .reshape(nh, NG, 128, G * 256)
        )
        return np.ascontiguousarray(t.astype(bf))

    wq_pk = w_pack(np.asarray(Wq, f).T * np.float32(SCALE))
    wk_pk = w_pack(np.asarray(Wk, f).T)
    wv_pk = w_pack(np.asarray(Wv, f).T)
    wp_pk = w_pack(np.asarray(Wp, f).T)  # 32 pseudo-heads of 256 jrows

    bias = np.ascontiguousarray(np.asarray(bp, f).reshape(64, 128).T)  # [128, 64]

    return xt, at, wq_pk, wk_pk, wv_pk, wp_pk, bias


def kernel(x, Augementation_embedding, Wq, Wk, Wv, Wp, bp):
    nc = _get_program()
    xt, at, wq_pk, wk_pk, wv_pk, wp_pk, bias = _pack_inputs(
        x, Augementation_embedding, Wq, Wk, Wv, Wp, bp
    )

    in_maps = []
    for c in range(NCORES):
        in_maps.append(
            {
                "xt": xt[c],
                "at": at[c],
                "wq": wq_pk,
                "wk": wk_pk,
                "wv": wv_pk,
                "wp": wp_pk,
                "bias": bias,
            }
        )

    res = run_bass_kernel_spmd(nc, in_maps, core_ids=list(range(NCORES)))
    outs = [res.results[c]["out"] for c in range(NCORES)]
    return np.concatenate(outs, axis=0).astype(np.float32)


if __name__ == "__main__":
    rng = np.random.default_rng(0)
    ins = {
        "x": rng.standard_normal((B, N, D), dtype=np.float32),
        "Augementation_embedding": rng.standard_normal((B, N, D), dtype=np.float32),
        "Wq": rng.standard_normal((D, D), dtype=np.float32) / np.sqrt(D),
        "Wk": rng.standard_normal((D, D), dtype=np.float32) / np.sqrt(D),
        "Wv": rng.standard_normal((D, D), dtype=np.float32) / np.sqrt(D),
        "Wp": rng.standard_normal((2 * D, D), dtype=np.float32) / np.sqrt(D),
        "bp": (rng.standard_normal(2 * D, dtype=np.float32) * 0.01),
    }
    o = kernel(**ins)
    print("out", o.shape, o.dtype, float(np.abs(o).max()))
